# revision 1
# baseline (speedup 1.0000x reference)
"""Bass/Tile Trainium2 kernel for a single attention head.

Problem: B=4, S=4096, D_IN=1024, D=128.
  q = query @ Wq + bq ; k = key @ Wk + bk ; v = value @ Wv + bv
  out = softmax(q k^T / sqrt(D)) v

Sharding: 8 cores; core c handles batch b=c//2, half h=c%2: it owns
query/key/value rows [h*2048, (h+1)*2048) of batch b. Each core projects
its own K/V half, the core pair exchanges projected K^T / V via a
pairwise AllGather (1 MiB), and each core then runs attention for its
2048 queries over all 4096 keys.

Math notes:
 - softmax over keys is invariant to per-query-row constants, so the
   bk bias term is dropped ((q+bq)@(k+bk)^T differs from (q+bq)@k^T by
   a per-row constant).
 - logits are small (|logit| < ~3 for randn inputs), so exp() without
   max-subtraction is numerically safe.
 - A ones-column appended to V gives the softmax denominator in the
   same PSUM accumulation as P@V.

Layout: matmuls contract over the partition dim, so q/k are kept
transposed ([d, seq]); inputs are transposed on the PE via
identity-matmuls. Scores are computed transposed (S^T[k, q]); exp runs
on the scalar engine PSUM->SBUF; P^T chunks serve directly as matmul
stationary for the AV product in natural [q, d] layout.
"""

import math
import sys

import numpy as np

for _p in ("/opt/trn_rl_repo", "/root/.axon_site/_ro/trn_rl_repo"):
    if _p not in sys.path:
        sys.path.append(_p)

import concourse.bass as bass  # noqa: E402
import concourse.mybir as mybir  # noqa: E402
import concourse.tile as tile  # noqa: E402
from concourse import bacc  # noqa: E402
from concourse.bass_utils import run_bass_kernel_spmd  # noqa: E402
from concourse.masks import make_identity  # noqa: E402

FP32 = mybir.dt.float32
BF16 = mybir.dt.bfloat16

B, S, D_IN, D = 4, 4096, 1024, 128
N_CORES = 8
VSLOT = 132  # per-key-tile slot width for v_aug (128 v cols + 1 ones + pad)


def build_program(nc, sq, skv_local, n_cores=8, pair_split=True, reps=1):
    """Emit the Tile program.

    sq: query rows per core. skv_local: kv rows this core projects.
    pair_split: exchange projected K/V across core pairs via AllGather
    (total keys = 2*skv_local); otherwise each core handles skv_local
    keys standalone.
    reps > 1 wraps the whole computation in an on-device For_i loop for
    benchmarking (only valid with pair_split=False: collectives cannot
    sit inside control flow).
    """
    assert reps == 1 or pair_split in (False, "mock")
    skv_tot = 2 * skv_local if pair_split else skv_local

    q_in = nc.dram_tensor("q_in", [sq, D_IN], FP32, kind="ExternalInput")
    k_in = nc.dram_tensor("k_in", [skv_local, D_IN], FP32, kind="ExternalInput")
    v_in = nc.dram_tensor("v_in", [skv_local, D_IN], FP32, kind="ExternalInput")
    wq = nc.dram_tensor("wq", [D_IN, D], FP32, kind="ExternalInput")
    wk = nc.dram_tensor("wk", [D_IN, D], FP32, kind="ExternalInput")
    wv = nc.dram_tensor("wv", [D_IN, D], FP32, kind="ExternalInput")
    bq = nc.dram_tensor("bq", [D, 1], FP32, kind="ExternalInput")
    bv = nc.dram_tensor("bv", [D, 1], FP32, kind="ExternalInput")
    out = nc.dram_tensor("out", [sq, D], FP32, kind="ExternalOutput")

    n_ic = D_IN // 128  # contraction chunks
    nkt_loc = skv_local // 128
    nkt = skv_tot // 128
    scale = 1.0 / math.sqrt(D)

    with tile.TileContext(nc) as tc:
        with (
            tc.tile_pool(name="const", bufs=1) as cpool,
            tc.tile_pool(name="wts", bufs=1) as wpool,
            tc.tile_pool(name="projout", bufs=1) as opool,
            tc.tile_pool(name="xload", bufs=3) as xpool,
            tc.tile_pool(name="xt", bufs=16) as xtpool,
            tc.tile_pool(name="ps", bufs=2, space="PSUM") as pspool,
            tc.tile_pool(name="av", bufs=4, space="PSUM") as avpool,
            tc.tile_pool(name="pt", bufs=6) as ptpool,
            tc.tile_pool(name="fin", bufs=4) as finpool,
            tc.tile_pool(name="dram", bufs=1, space="DRAM") as dpool,
        ):
            def emit_body():
                ident = cpool.tile([128, 128], BF16)
                make_identity(nc, ident[:])

                # Each W [1024, 128] loads as one cast-DMA into a [128, 8*128]
                # tile; chunk ic lives at cols ic*128:(ic+1)*128 with the
                # contraction index on partitions.
                w_sb = {}
                for name, wdram in (("wq", wq), ("wk", wk), ("wv", wv)):
                    t = wpool.tile([128, n_ic * D], BF16, tag=f"w_{name}")
                    nc.gpsimd.dma_start(
                        out=t[:].rearrange("p (c d) -> p c d", c=n_ic),
                        in_=wdram[:, :].rearrange("(c p) d -> p c d", c=n_ic),
                    )
                    w_sb[name] = [t[:, ic * D : (ic + 1) * D] for ic in range(n_ic)]

                bq_sb = cpool.tile([128, 1], FP32, tag="bq")
                nc.sync.dma_start(out=bq_sb[:], in_=bq[:, :])
                bv_sb = cpool.tile([128, 1], FP32, tag="bv")
                nc.sync.dma_start(out=bv_sb[:], in_=bv[:, :])

                qT = opool.tile([128, sq], BF16, tag="qT")
                kTl = opool.tile([128, skv_local], BF16, tag="kTl")
                vTl = opool.tile([128, skv_local], BF16, tag="vTl")
                vtl = opool.tile([128, nkt_loc * 128], BF16, tag="vtl")
                kT = opool.tile([128, skv_tot], BF16, tag="kT")
                vfull = opool.tile([128, nkt * VSLOT], BF16, tag="vfull")

                def kslice(kt):
                    if not pair_split:
                        return kTl[:, kt * 128 : (kt + 1) * 128]
                    return kT[:, kt * 128 : (kt + 1) * 128]

                def project(x_dram, s_len, w_tiles, dstT, bias_ap):
                    for sb in range(s_len // 512):
                        # one cast-DMA per 512-row block; subblock ss sits at
                        # cols ss*D_IN:(ss+1)*D_IN with rows on partitions
                        xs = xpool.tile([128, 4 * D_IN], BF16, tag="xload")
                        r0 = sb * 512
                        nc.gpsimd.dma_start(
                            out=xs[:].rearrange("p (s i) -> p s i", s=4),
                            in_=x_dram[r0 : r0 + 512, :].rearrange(
                                "(s p) i -> p s i", s=4
                            ),
                        )
                        xts = []
                        for icp in range(n_ic // 2):
                            # two contraction chunks share one PSUM bank
                            # (bf16 [128,1024] = 2KB/partition) -> one copy
                            tp = pspool.tile([128, 1024], BF16, tag="ps")
                            for half in range(2):
                                ic = 2 * icp + half
                                for ss in range(4):
                                    nc.tensor.transpose(
                                        tp[
                                            :,
                                            half * 512
                                            + ss * 128 : half * 512
                                            + (ss + 1) * 128,
                                        ],
                                        xs[
                                            :,
                                            ss * D_IN
                                            + ic * 128 : ss * D_IN
                                            + (ic + 1) * 128,
                                        ],
                                        ident[:],
                                    )
                            xt_sb = xtpool.tile([128, 1024], BF16, tag="xt")
                            nc.vector.tensor_copy(xt_sb[:], tp[:])
                            xts.append(xt_sb[:, 0:512])
                            xts.append(xt_sb[:, 512:1024])
                        pp = pspool.tile([128, 512], FP32, tag="ps")
                        for ic in range(n_ic):
                            nc.tensor.matmul(
                                pp[:],
                                w_tiles[ic],
                                xts[ic],
                                start=(ic == 0),
                                stop=(ic == n_ic - 1),
                            )
                        dst = dstT[:, sb * 512 : (sb + 1) * 512]
                        if bias_ap is None:
                            nc.vector.tensor_copy(dst, pp[:])
                        else:
                            nc.vector.tensor_scalar_add(dst, pp[:], bias_ap)

                # K/V first so the pair exchange overlaps the Q-side work.
                project(k_in, skv_local, w_sb["wk"], kTl, None)
                project(v_in, skv_local, w_sb["wv"], vTl, bv_sb[:])

                # local v natural tiles from vT
                for kt in range(nkt_loc):
                    tp = pspool.tile([128, 512], BF16, tag="ps")
                    nc.tensor.transpose(
                        tp[:, 0:128], vTl[:, kt * 128 : (kt + 1) * 128], ident[:]
                    )
                    nc.vector.tensor_copy(vtl[:, kt * 128 : (kt + 1) * 128], tp[:, 0:128])

                if pair_split:
                    groups = [[2 * i, 2 * i + 1] for i in range(n_cores // 2)]
                    cc_in = dpool.tile([2, 128, skv_local], BF16, tag="cc_in")
                    cc_out = dpool.tile([2, 2, 128, skv_local], BF16, tag="cc_out")
                    nc.sync.dma_start(out=cc_in[0], in_=kTl[:])
                    nc.sync.dma_start(out=cc_in[1], in_=vtl[:])
                    if pair_split == "mock":
                        # benchmarking stand-in: same buffers and data volume,
                        # but a local DRAM round-trip instead of the AllGather
                        # (wrong remote-half numerics; timing only)
                        nc.sync.dma_start(out=cc_out[0], in_=cc_in[:])
                        nc.sync.dma_start(out=cc_out[1], in_=cc_in[:])
                    else:
                        nc.gpsimd.collective_compute(
                            "AllGather",
                            mybir.AluOpType.bypass,
                            replica_groups=groups,
                            ins=[cc_in.opt()],
                            outs=[cc_out.opt()],
                        )

                if pair_split:
                    # unpack gathered halves: rank order == key order
                    for h in range(2):
                        nc.sync.dma_start(
                            out=kT[:, h * skv_local : (h + 1) * skv_local],
                            in_=cc_out[h, 0],
                        )
                        nc.sync.dma_start(
                            out=vfull[
                                :, h * nkt_loc * VSLOT : (h + 1) * nkt_loc * VSLOT
                            ].rearrange("p (j s) -> p j s", j=nkt_loc)[:, :, 0:128],
                            in_=cc_out[h, 1].rearrange("p (j d) -> p j d", j=nkt_loc),
                        )
                else:
                    nc.vector.tensor_copy(
                        vfull[:, 0 : nkt_loc * VSLOT].rearrange(
                            "p (j s) -> p j s", j=nkt_loc
                        )[:, :, 0:128],
                        vtl[:].rearrange("p (j d) -> p j d", j=nkt_loc),
                    )

                project(q_in, sq, w_sb["wq"], qT, bq_sb[:])

                for kt in range(nkt):
                    nc.vector.memset(
                        vfull[:, kt * VSLOT + 128 : kt * VSLOT + 129], 1.0
                    )

                # attention: scores^T -> exp -> (P^T)^T @ v_aug
                for qb in range(sq // 512):
                    avs = []
                    for qs in range(4):
                        av_t = avpool.tile([128, VSLOT], FP32, tag="av")
                        avs.append(av_t)
                    for ktp in range(nkt // 2):
                        # two key tiles per PSUM allocation (2 banks) and one
                        # exp() over both -> half the ACT instruction count
                        sc = pspool.tile([128, 1024], FP32, tag="ps")
                        for half in range(2):
                            kt = 2 * ktp + half
                            nc.tensor.matmul(
                                sc[:, half * 512 : (half + 1) * 512],
                                kslice(kt),
                                qT[:, qb * 512 : (qb + 1) * 512],
                                start=True,
                                stop=True,
                            )
                        pt = ptpool.tile([128, 1024], BF16, tag="pt")
                        nc.scalar.activation(
                            pt[:],
                            sc[:],
                            mybir.ActivationFunctionType.Exp,
                            bias=0.0,
                            scale=scale,
                        )
                        for half in range(2):
                            kt = 2 * ktp + half
                            for qs in range(4):
                                nc.tensor.matmul(
                                    avs[qs][:, 0:129],
                                    pt[
                                        :,
                                        half * 512
                                        + qs * 128 : half * 512
                                        + (qs + 1) * 128,
                                    ],
                                    vfull[:, kt * VSLOT : kt * VSLOT + 129],
                                    start=(kt == 0),
                                    stop=(kt == nkt - 1),
                                )
                    obuf = finpool.tile([128, 4 * D], FP32, tag="obuf")
                    for qs in range(4):
                        rec = finpool.tile([128, 1], FP32, tag="rec")
                        nc.vector.reciprocal(rec[:], avs[qs][:, 128:129])
                        nc.vector.tensor_scalar_mul(
                            obuf[:, qs * D : (qs + 1) * D], avs[qs][:, 0:128], rec[:]
                        )
                    r0 = qb * 512
                    nc.sync.dma_start(
                        out=out[r0 : r0 + 512, :].rearrange("(s p) d -> p s d", s=4),
                        in_=obuf[:].rearrange("p (s d) -> p s d", s=4),
                    )

            if reps > 1:
                hint = (
                    mybir.EngineType.PE,
                    mybir.EngineType.DVE,
                    mybir.EngineType.Activation,
                    mybir.EngineType.SP,
                    mybir.EngineType.Pool,
                )
                with tc.For_i(0, reps, 1, hint_engines=hint):
                    emit_body()
            else:
                emit_body()

    return nc


def build_graph(
    sq=S // 2, skv_local=S // 2, n_cores=N_CORES, pair_split=True, reps=1
):
    nc = bacc.Bacc(
        "TRN2",
        target_bir_lowering=False,
        debug=False,
        enable_asserts=True,
        num_devices=n_cores,
    )
    build_program(
        nc, sq, skv_local, n_cores=n_cores, pair_split=pair_split, reps=reps
    )
    nc.compile()
    return nc


_NC = None


def _get_nc():
    global _NC
    if _NC is None:
        _NC = build_graph()
    return _NC


def make_in_maps(query, key, value, Wq, bq, Wk, bk, Wv, bv):
    query = np.asarray(query, dtype=np.float32)
    key = np.asarray(key, dtype=np.float32)
    value = np.asarray(value, dtype=np.float32)
    Wq = np.ascontiguousarray(np.asarray(Wq, dtype=np.float32))
    Wk = np.ascontiguousarray(np.asarray(Wk, dtype=np.float32))
    Wv = np.ascontiguousarray(np.asarray(Wv, dtype=np.float32))
    bq2 = np.ascontiguousarray(np.asarray(bq, np.float32).reshape(D, 1))
    bv2 = np.ascontiguousarray(np.asarray(bv, np.float32).reshape(D, 1))
    sq = S // 2
    in_maps = []
    for c in range(N_CORES):
        b, h = c // 2, c % 2
        in_maps.append(
            {
                "q_in": np.ascontiguousarray(query[b, h * sq : (h + 1) * sq, :]),
                "k_in": np.ascontiguousarray(key[b, h * sq : (h + 1) * sq, :]),
                "v_in": np.ascontiguousarray(value[b, h * sq : (h + 1) * sq, :]),
                "wq": Wq,
                "wk": Wk,
                "wv": Wv,
                "bq": bq2,
                "bv": bv2,
            }
        )
    return in_maps


def assemble_out(results):
    sq = S // 2
    out = np.empty((B, S, D), np.float32)
    for c in range(N_CORES):
        b, h = c // 2, c % 2
        out[b, h * sq : (h + 1) * sq, :] = results[c]["out"]
    return out


def kernel(query, key, value, Wq, bq, Wk, bk, Wv, bv):
    nc = _get_nc()
    in_maps = make_in_maps(query, key, value, Wq, bq, Wk, bk, Wv, bv)
    res = run_bass_kernel_spmd(nc, in_maps, core_ids=list(range(N_CORES)))
    return assemble_out(res.results)

